# revision 19
# baseline (speedup 1.0000x reference)
"""Trainium2 Bass kernel for a dense GQA transformer block (B=1, T=2048, C=2048,
16 q heads / 8 kv heads, hs=128, SwiGLU FFN=5632), SPMD across 8 NeuronCores.

Sharding: tensor-parallel attention (2 q heads + 1 kv head per core, full T),
one AllToAll to re-shard from head-parallel to row-parallel, then the attn
projection, residual, norm2 and the whole MLP run row-parallel (256 rows/core,
full weights streamed from HBM as bf16). Only collective: one 1MB AllToAll.

All activations are kept feature-major ("c-major" [C, T] / [HS, T]) so every
matmul maps directly onto the PE array without transposes; cross-feature
reductions (rms-norm sums, softmax sums) use ones-vector matmuls, and
partition-broadcasts use K=1 ones matmuls (reciprocals run post-broadcast on
[128, N] tiles — 1-partition DVE ops are ~8x slower).

Compute dtype: bf16 inputs to the PE with fp32 PSUM accumulation.
"""

import numpy as np
import ml_dtypes

N_CORES = 8
T = 2048
C = 2048
NH = 16
NKV = 8
HS = 128
FFN = 5632
EPS = 1e-5
R = T // N_CORES          # 256 rows (tokens) per core after the A2A
NCB = C // 128            # 16 feature blocks
NFB = FFN // 128          # 44 FFN blocks
NTCH = T // 512           # 4 T-chunks of 512
SM_SCALE = 1.0 / np.sqrt(np.float32(HS))
SM_BIAS = -10.0           # softmax exp bias; max |score| measured ~7, f32 exp safe
BF16 = ml_dtypes.bfloat16

_CACHE = {}
LDW_OPT = False


def _patch_ldw_opt():
    # bass_utils hardcodes --enable-ldw-opt=false; LDWEIGHTS at ~120ns paces
    # our N=256 matmuls. Rewrite the flag on the walrus command line.
    if not LDW_OPT or _CACHE.get("ldw_patched"):
        return
    from concourse import bass_utils
    orig = bass_utils.run_command

    def run_command(argv, **kwargs):
        argv = [a.replace("--enable-ldw-opt=false", "--enable-ldw-opt=true")
                if isinstance(a, str) else a for a in argv]
        return orig(argv, **kwargs)

    bass_utils.run_command = run_command
    _CACHE["ldw_patched"] = True


def _build():
    _patch_ldw_opt()
    import concourse.mybir as mybir
    import concourse.tile as tile
    from concourse import bacc

    f32 = mybir.dt.float32
    bf16 = mybir.dt.bfloat16
    Exp = mybir.ActivationFunctionType.Exp
    Silu = mybir.ActivationFunctionType.Silu
    Sqrt = mybir.ActivationFunctionType.Sqrt

    nc = bacc.Bacc(trn_type="TRN2", num_devices=N_CORES)

    # ---- kernel I/O (all host-pre-arranged to partition-major layouts) ----
    xP = nc.dram_tensor("xP", [128, NCB * T], bf16, kind="ExternalInput")
    cosT = nc.dram_tensor("cosT", [128, T], bf16, kind="ExternalInput")
    sinT = nc.dram_tensor("sinT", [128, T], bf16, kind="ExternalInput")
    # qkv weight tiles: [p, (db*16+cb)*128+f], db: 0=q0 1=q1 2=k 3=v
    wqkv = nc.dram_tensor("wqkv", [128, 64 * 128], bf16, kind="ExternalInput")
    # attn proj tiles per cout block: [co][p, yb*128+f]
    wproj = nc.dram_tensor("wproj", [16, 128, 16 * 128], bf16, kind="ExternalInput")
    # fc1|fc2 tiles per FFN block: [fb][p, (s*16+cb)*128+f]
    w12 = nc.dram_tensor("w12", [NFB, 128, 2 * 16 * 128], bf16, kind="ExternalInput")
    # mlp proj tiles per cout block: [co][p, fb*128+f]
    w3 = nc.dram_tensor("w3", [16, 128, NFB * 128], bf16, kind="ExternalInput")
    # residual x rows (this core's R tokens), c-major: [p, co*R+t]
    xrows = nc.dram_tensor("xrows", [128, 16 * R], f32, kind="ExternalInput")
    outT = nc.dram_tensor("outT", [C, R], f32, kind="ExternalOutput")

    with tile.TileContext(nc) as tc:
        with (
            tc.tile_pool(name="const", bufs=1) as constp,
            tc.tile_pool(name="dram", bufs=1, space="DRAM") as dramp,
            tc.tile_pool(name="w12s", bufs=8) as w12p,
            tc.tile_pool(name="qkv_acts", bufs=1) as qvp,
        ):
            # ---------------- constants ----------------
            ones_col = constp.tile([128, 1], bf16)
            nc.vector.memset(ones_col, 1.0)
            ones_row = constp.tile([1, 128], f32)
            nc.vector.memset(ones_row, 1.0)
            eps_t = constp.tile([128, 1], f32)
            nc.vector.memset(eps_t, EPS)
            smbias_t = constp.tile([128, 1], f32)
            nc.vector.memset(smbias_t, SM_BIAS)
            masks = constp.tile([128, 4 * 512], bf16)
            nc.vector.memset(masks, 1.0)
            for j in range(4):
                # keep 1 where tq >= tk + 128*j  (iota = -x + y - 128j >= 0)
                nc.gpsimd.affine_select(
                    out=masks[:, j * 512:(j + 1) * 512],
                    in_=masks[:, j * 512:(j + 1) * 512],
                    compare_op=mybir.AluOpType.is_ge,
                    fill=0.0,
                    base=-128 * j,
                    pattern=[[1, 512]],
                    channel_multiplier=-1,
                )

            # a2a buffers (one collective per local head, fired as each
            # head's attention completes -> hides trigger latency + core skew)
            a2a_in0 = dramp.tile([8 * 128, R], bf16)
            a2a_out0 = dramp.tile([8 * 128, R], bf16)
            a2a_in1 = dramp.tile([8 * 128, R], bf16)
            a2a_out1 = dramp.tile([8 * 128, R], bf16)

            qk_sb = qvp.tile([128, 3 * T], bf16)     # roped q0|q1|k, d-major
            v_sb = qvp.tile([128, NCB * 128], bf16)  # v token-major tiles

            with (
                tc.tile_pool(name="cs", bufs=1) as csp,
                tc.tile_pool(name="wqp", bufs=1) as wqpool,
                tc.tile_pool(name="xbfp", bufs=1) as xbfp,
            ):
                cs_sb = csp.tile([128, 2 * T], bf16)
                cosl = cs_sb[0:64, 0:T]
                cosh = cs_sb[64:128, 0:T]
                sinl = cs_sb[0:64, T:2 * T]
                sinh = cs_sb[64:128, T:2 * T]
                wq_sb = wqpool.tile([128, 64 * 128], bf16)
                xn = xbfp.tile([128, NCB * T], bf16)

                # ================= phase 1: rms-norm 1 =================
                with (
                    nc.named_scope("norm1"),
                    tc.tile_pool(name="xsq", bufs=3) as sqp,
                    tc.tile_pool(name="n1small", bufs=1) as n1s,
                    tc.tile_pool(name="ps_ss", bufs=1, space="PSUM") as pss,
                    tc.tile_pool(name="ps_bc", bufs=2, space="PSUM") as psb,
                ):
                    ss_ps = [pss.tile([1, 512], f32, name=f"ss{t4}", tag=f"ss{t4}")
                             for t4 in range(NTCH)]
                    for cb in range(NCB):
                        xsl = xn[:, cb * T:(cb + 1) * T]
                        nc.sync.dma_start(xsl, xP[:, cb * T:(cb + 1) * T])
                        xsq = sqp.tile([128, T], bf16, tag="xsq")
                        nc.vector.tensor_mul(xsq[:], xsl, xsl)
                        for t4 in range(NTCH):
                            nc.tensor.matmul(
                                ss_ps[t4][:], ones_col[:],
                                xsq[:, t4 * 512:(t4 + 1) * 512],
                                start=(cb == 0), stop=(cb == NCB - 1))
                    sg = n1s.tile([1, T], f32)
                    rbc = n1s.tile([128, T], bf16)
                    for t4 in range(NTCH):
                        ch = slice(t4 * 512, (t4 + 1) * 512)
                        nc.scalar.activation(sg[:, ch], ss_ps[t4][:],
                                             Sqrt, bias=eps_t[0:1, :], scale=1.0 / C)
                        bc = psb.tile([128, 512], f32, tag="bc")
                        nc.tensor.matmul(bc[:], ones_row[:], sg[:, ch],
                                         start=True, stop=True)
                        rtmp = n1s.tile([128, 512], f32, tag="rtmp", bufs=2)
                        nc.vector.reciprocal_approx_fast(out=rtmp[:], in_=bc[:])
                        nc.vector.tensor_copy(rbc[:, ch], rtmp[:])
                    for cb in range(NCB):
                        nc.vector.tensor_mul(xn[:, cb * T:(cb + 1) * T],
                                             xn[:, cb * T:(cb + 1) * T], rbc[:])

                # qkv weights + rope tables load behind the x stream
                for db in range(4):
                    nc.sync.dma_start(wq_sb[:, db * 2048:(db + 1) * 2048],
                                      wqkv[:, db * 2048:(db + 1) * 2048])
                nc.sync.dma_start(cs_sb[:, 0:T], cosT[:])
                nc.sync.dma_start(cs_sb[:, T:2 * T], sinT[:])

                # ================= phase 2: qkv + rope =================
                with (
                    nc.named_scope("qkv"),
                    tc.tile_pool(name="ropetmp", bufs=6) as rtp,
                    tc.tile_pool(name="ps_qk", bufs=3, space="PSUM") as psqk,
                    tc.tile_pool(name="ps_v", bufs=2, space="PSUM") as psv,
                ):
                    for db in range(3):  # q0, q1, k -> d-major
                        for t4 in range(NTCH):
                            qp = psqk.tile([128, 512], f32, tag="qk")
                            for cb in range(NCB):
                                nc.tensor.matmul(
                                    qp[:],
                                    wq_sb[:, (db * 16 + cb) * 128:
                                          (db * 16 + cb + 1) * 128],
                                    xn[:, cb * T + t4 * 512: cb * T + (t4 + 1) * 512],
                                    start=(cb == 0), stop=(cb == NCB - 1))
                            # rope into qk_sb[:, db*T + chunk]. SBUF-SBUF DVE ops
                            # need equal base partitions; PSUM inputs are exempt,
                            # so crossed-half terms read q straight from PSUM.
                            ch = slice(t4 * 512, (t4 + 1) * 512)
                            dst = qk_sb[:, db * T + t4 * 512: db * T + (t4 + 1) * 512]
                            rc = rtp.tile([128, 512], bf16, tag="rc")
                            nc.vector.tensor_mul(rc[:], qp[:], cs_sb[:, 0:T][:, ch])
                            cross = rtp.tile([128, 512], bf16, tag="cross")
                            nc.vector.tensor_mul(cross[0:64, :], qp[64:128, :],
                                                 sinl[:, ch])
                            nc.vector.tensor_mul(cross[64:128, :], qp[0:64, :],
                                                 sinh[:, ch])
                            nc.vector.tensor_sub(dst[0:64, :], rc[0:64, :],
                                                 cross[0:64, :])
                            nc.vector.tensor_add(dst[64:128, :], rc[64:128, :],
                                                 cross[64:128, :])
                    for tb_ in range(NCB):  # v token-major
                        vp = psv.tile([128, 128], f32, tag="v")
                        for cb in range(NCB):
                            nc.tensor.matmul(
                                vp[:],
                                xn[:, cb * T + tb_ * 128: cb * T + (tb_ + 1) * 128],
                                wq_sb[:, (48 + cb) * 128:(48 + cb + 1) * 128],
                                start=(cb == 0), stop=(cb == NCB - 1))
                        nc.vector.tensor_copy(
                            v_sb[:, tb_ * 128:(tb_ + 1) * 128], vp[:])

            # x / cos / qkv-weight buffers freed here; MLP+proj weights stream in.
            # allocate all MLP fc weight tiles now so their DMAs start during attn
            w12_tiles = {}
            for fb in range(NFB):
                w12_tiles[fb] = w12p.tile(
                    [128, 2 * 16 * 128], bf16, name=f"w12t{fb}", tag="w12t")
                nc.sync.dma_start(w12_tiles[fb][:], w12[fb])

            with (
                tc.tile_pool(name="wprojs", bufs=6) as projp,
                tc.tile_pool(name="late", bufs=1) as latep,
            ):
                proj_tiles = []
                for co in range(16):
                    wt = projp.tile([128, 16 * 128], bf16, name=f"projw{co}",
                                    tag="projw")
                    nc.sync.dma_start(wt[:], wproj[co])
                    proj_tiles.append(wt)

                x2_sb = latep.tile([128, 16 * R], f32)
                xn2_sb = latep.tile([128, 16 * R], bf16)
                h_sb = latep.tile([128, NFB * R], bf16)
                y_all = latep.tile([128, 16 * R], bf16)

                # ============ phase 3: attention (2 heads per core) ============
                with (
                    nc.named_scope("attn"),
                    tc.tile_pool(name="pp_p", bufs=12) as ppool,
                    tc.tile_pool(name="pp_y", bufs=4) as ypool,
                    tc.tile_pool(name="attn_small", bufs=4) as asml,
                    tc.tile_pool(name="ps_s", bufs=3, space="PSUM") as ps_s,
                    tc.tile_pool(name="ps_y", bufs=2, space="PSUM") as ps_y,
                    tc.tile_pool(name="ps_sum", bufs=2, space="PSUM") as ps_sum,
                    tc.tile_pool(name="ps_abc", bufs=1, space="PSUM") as ps_abc,
                ):
                    for h in range(2):
                        a2a_in_h = a2a_in0 if h == 0 else a2a_in1
                        q_ap = qk_sb[:, h * T:(h + 1) * T]
                        k_ap = qk_sb[:, 2 * T:3 * T]
                        for qi in range(NTCH):
                            nkb = 4 * qi + 4
                            yp = ps_y.tile([128, 512], f32, tag="y")
                            sacc = asml.tile([128, 512], f32, tag="sacc")
                            pend = []  # 2-deep SW pipeline: AV trails scores
                            ptiles = {}

                            def flush(kb):
                                ppt = ptiles[kb]
                                nc.tensor.matmul(
                                    yp[:], v_sb[:, kb * 128:(kb + 1) * 128],
                                    ppt[:], start=(kb == 0), stop=(kb == nkb - 1))
                                if kb == 0:
                                    nc.vector.tensor_copy(sacc[:], ppt[:])
                                else:
                                    nc.vector.tensor_add(sacc[:], sacc[:], ppt[:])

                            for kb in range(nkb):
                                sp = ps_s.tile([128, 512], f32, tag="s")
                                nc.tensor.matmul(
                                    sp[:], k_ap[:, kb * 128:(kb + 1) * 128],
                                    q_ap[:, qi * 512:(qi + 1) * 512],
                                    start=True, stop=True)
                                pt = ppool.tile([128, 512], bf16, tag="p")
                                nc.scalar.activation(pt[:], sp[:], Exp,
                                                     bias=smbias_t[:],
                                                     scale=float(SM_SCALE))
                                if kb >= 4 * qi:
                                    moff = kb - 4 * qi
                                    nc.vector.tensor_mul(
                                        pt[:], pt[:],
                                        masks[:, moff * 512:(moff + 1) * 512])
                                ptiles[kb] = pt
                                pend.append(kb)
                                if len(pend) > 4:
                                    flush(pend.pop(0))
                            while pend:
                                flush(pend.pop(0))
                            # S = column-sums of accumulated p, then 1/S bcast
                            sump = ps_sum.tile([1, 512], f32, tag="sum")
                            sacc_bf = asml.tile([128, 512], bf16, tag="saccb")
                            nc.vector.tensor_copy(sacc_bf[:], sacc[:])
                            nc.tensor.matmul(sump[:], ones_col[:], sacc_bf[:],
                                             start=True, stop=True)
                            ssb = asml.tile([1, 512], f32, tag="ssb")
                            nc.vector.tensor_copy(ssb[:], sump[:])
                            bcp = ps_abc.tile([128, 512], f32, tag="abc")
                            nc.tensor.matmul(bcp[:], ones_row[:], ssb[:],
                                             start=True, stop=True)
                            bsb = asml.tile([128, 512], f32, tag="bsb")
                            nc.vector.reciprocal_approx_fast(out=bsb[:], in_=bcp[:])
                            ysb = ypool.tile([128, 512], bf16, tag="ysb")
                            nc.vector.tensor_mul(ysb[:], yp[:], bsb[:])
                            # scatter two 256-token halves to this head's A2A buf
                            for half in range(2):
                                g = 2 * qi + half
                                nc.sync.dma_start(
                                    a2a_in_h[128 * g: 128 * (g + 1), :],
                                    ysb[:, half * 256:(half + 1) * 256])
                        # fire this head's A2A as soon as its outputs are staged,
                        # and pull the result into SBUF immediately
                        a2a_out_h = a2a_out0 if h == 0 else a2a_out1
                        nc.gpsimd.collective_compute(
                            "AllToAll", mybir.AluOpType.bypass,
                            replica_groups=[list(range(N_CORES))],
                            ins=[a2a_in_h.opt()], outs=[a2a_out_h.opt()])
                        for g in range(8):
                            nc.sync.dma_start(
                                y_all[:, (h * 8 + g) * R:(h * 8 + g + 1) * R],
                                a2a_out_h[g * 128:(g + 1) * 128, :])

                # ======== phase 5: proj + residual + norm2 (row-local) ========
                with (
                    nc.named_scope("proj"),
                    tc.tile_pool(name="sq2", bufs=2) as sq2p,
                    tc.tile_pool(name="xrow", bufs=1) as xrp,
                    tc.tile_pool(name="n2small", bufs=1) as n2s,
                    tc.tile_pool(name="ps_acc", bufs=2, space="PSUM") as psa,
                    tc.tile_pool(name="ps_ss2", bufs=1, space="PSUM") as pss2,
                    tc.tile_pool(name="ps_bc2", bufs=1, space="PSUM") as psb2,
                ):
                    xr_sb = xrp.tile([128, 16 * R], f32)
                    nc.scalar.dma_start(xr_sb[:], xrows[:])
                    ss2 = pss2.tile([1, R], f32, tag="ss2")
                    for co in range(16):
                        wt = proj_tiles[co]
                        ap = psa.tile([128, R], f32, tag="acc")
                        for yb in range(16):
                            nc.tensor.matmul(ap[:], wt[:, yb * 128:(yb + 1) * 128],
                                             y_all[:, yb * R:(yb + 1) * R],
                                             start=(yb == 0), stop=(yb == 15))
                        cs_ = slice(co * R, (co + 1) * R)
                        nc.vector.tensor_add(x2_sb[:, cs_], ap[:], xr_sb[:, cs_])
                        xq2 = sq2p.tile([128, R], bf16, tag="xq2")
                        nc.vector.tensor_mul(xq2[:], x2_sb[:, cs_], x2_sb[:, cs_])
                        nc.tensor.matmul(ss2[:], ones_col[:], xq2[:],
                                         start=(co == 0), stop=(co == 15))
                    sg2 = n2s.tile([1, R], f32)
                    nc.scalar.activation(sg2[:], ss2[:], Sqrt,
                                         bias=eps_t[0:1, :], scale=1.0 / C)
                    bc2 = psb2.tile([128, R], f32, tag="bc2")
                    nc.tensor.matmul(bc2[:], ones_row[:], sg2[:],
                                     start=True, stop=True)
                    b2sb = n2s.tile([128, R], f32)
                    nc.vector.reciprocal_approx_fast(out=b2sb[:], in_=bc2[:])
                    for co in range(16):
                        cs_ = slice(co * R, (co + 1) * R)
                        nc.vector.tensor_mul(xn2_sb[:, cs_], x2_sb[:, cs_], b2sb[:])

                # ================= phase 6a: MLP fc1/fc2 + swiglu =============
                with (
                    tc.tile_pool(name="w3s", bufs=3) as w3p,
                ):
                    w3_tiles = []
                    for co in range(16):
                        w3t = w3p.tile([128, NFB * 128], bf16, name=f"w3t{co}",
                                       tag="w3w")
                        nc.sync.dma_start(w3t[:], w3[co])
                        w3_tiles.append(w3t)
                    with (
                        nc.named_scope("mlp_fc"),
                        tc.tile_pool(name="hsil", bufs=2) as hsp,
                        tc.tile_pool(name="ps_h1", bufs=2, space="PSUM") as psh1,
                        tc.tile_pool(name="ps_h2", bufs=2, space="PSUM") as psh2,
                    ):
                        for fb in range(NFB):
                            wt = w12_tiles[fb]
                            h1 = psh1.tile([128, R], f32, tag="h1")
                            h2 = psh2.tile([128, R], f32, tag="h2")
                            for cb in range(16):
                                nc.tensor.matmul(
                                    h1[:], wt[:, cb * 128:(cb + 1) * 128],
                                    xn2_sb[:, cb * R:(cb + 1) * R],
                                    start=(cb == 0), stop=(cb == 15))
                            for cb in range(16):
                                nc.tensor.matmul(
                                    h2[:], wt[:, (16 + cb) * 128:(17 + cb) * 128],
                                    xn2_sb[:, cb * R:(cb + 1) * R],
                                    start=(cb == 0), stop=(cb == 15))
                            hs = hsp.tile([128, R], f32, tag="hs")
                            nc.scalar.activation(hs[:], h1[:], Silu)
                            nc.vector.tensor_mul(h_sb[:, fb * R:(fb + 1) * R],
                                                 hs[:], h2[:])

                    # ============== phase 6b: MLP proj + final residual =======
                    with (
                        nc.named_scope("mlp_proj"),
                        tc.tile_pool(name="outp", bufs=3) as outp,
                        tc.tile_pool(name="ps_o", bufs=2, space="PSUM") as pso,
                    ):
                        for co in range(16):
                            w3t = w3_tiles[co]
                            op = pso.tile([128, R], f32, tag="o")
                            for fb in range(NFB):
                                nc.tensor.matmul(
                                    op[:], w3t[:, fb * 128:(fb + 1) * 128],
                                    h_sb[:, fb * R:(fb + 1) * R],
                                    start=(fb == 0), stop=(fb == NFB - 1))
                            osb = outp.tile([128, R], f32, tag="osb")
                            nc.vector.tensor_add(osb[:], op[:],
                                                 x2_sb[:, co * R:(co + 1) * R])
                            nc.scalar.dma_start(outT[co * 128:(co + 1) * 128, :], osb[:])

    nc.compile()
    return nc


def _prep_inputs(inputs):
    """Host-side sharding / layout / dtype prep. Returns per-core in_maps."""
    x = np.asarray(inputs["x"], np.float32)[0]        # (T, C)
    cos = np.asarray(inputs["cos"], np.float32)[0]    # (T, HS)
    sin = np.asarray(inputs["sin"], np.float32)[0]
    qkv_w = np.asarray(inputs["qkv_w"], np.float32)   # (4096, C)
    proj_w = np.asarray(inputs["proj_w"], np.float32)  # (C, 2048)
    fc1_w = np.asarray(inputs["fc1_w"], np.float32)   # (FFN, C)
    fc2_w = np.asarray(inputs["fc2_w"], np.float32)
    mlp_proj_w = np.asarray(inputs["mlp_proj_w"], np.float32)  # (C, FFN)
    n1 = np.asarray(inputs["norm1_w"], np.float32)
    n2 = np.asarray(inputs["norm2_w"], np.float32)

    xT = np.ascontiguousarray(x.T)                    # (C, T)
    # xP[p, cb*T + t] = xT[cb*128+p, t]
    xP = np.ascontiguousarray(
        xT.reshape(NCB, 128, T).transpose(1, 0, 2).reshape(128, NCB * T)).astype(BF16)
    cosT = np.ascontiguousarray(cos.T).astype(BF16)
    sinT = np.ascontiguousarray(sin.T).astype(BF16)

    qkv_eff = (qkv_w * n1[None, :]).astype(BF16)      # fold norm1 weight
    # per-core d-major weight tiles
    wqkv_cores = []
    for i in range(N_CORES):
        dblocks = [
            qkv_eff[(2 * i) * HS:(2 * i + 1) * HS],       # q0
            qkv_eff[(2 * i + 1) * HS:(2 * i + 2) * HS],   # q1
            qkv_eff[NH * HS + i * HS: NH * HS + (i + 1) * HS],            # k
            qkv_eff[(NH + NKV) * HS + i * HS: (NH + NKV) * HS + (i + 1) * HS],  # v
        ]
        # tile (db, cb): lhsT[p, f] = W^T[cb*128+p, db*128+f] = W[db*128+f, cb*128+p]
        blocks = [dblocks[db].T.reshape(NCB, 128, 128) for db in range(4)]
        arr = np.stack(blocks, axis=0)              # (db, cb, p, f)
        wqkv_cores.append(np.ascontiguousarray(
            arr.transpose(2, 0, 1, 3).reshape(128, 64 * 128)))

    projT = proj_w.T.astype(BF16)                   # (ych, cout)
    # y_all channel-block order after the two per-head A2As: blocks 0-7 are the
    # even global heads (local head 0 of cores 0-7), 8-15 the odd ones.
    perm = [2 * g for g in range(8)] + [2 * g + 1 for g in range(8)]
    wproj = np.ascontiguousarray(
        projT.reshape(16, 128, 16, 128)[perm].transpose(2, 1, 0, 3)
        .reshape(16, 128, 16 * 128))

    w1T = (fc1_w * n2[None, :]).T.astype(BF16)      # (C, FFN)
    w2T = (fc2_w * n2[None, :]).T.astype(BF16)
    # w12[fb][p, (s*16+cb)*128+f] = wsT[cb*128+p, fb*128+f]
    a1 = w1T.reshape(NCB, 128, NFB, 128)            # (cb, p, fb, f)
    a2 = w2T.reshape(NCB, 128, NFB, 128)
    w12 = np.ascontiguousarray(
        np.stack([a1, a2], axis=0)                  # (s, cb, p, fb, f)
        .transpose(3, 2, 0, 1, 4)                   # (fb, p, s, cb, f)
        .reshape(NFB, 128, 2 * 16 * 128))
    mlpT = mlp_proj_w.T.astype(BF16)                # (FFN, C)
    w3 = np.ascontiguousarray(
        mlpT.reshape(NFB, 128, 16, 128).transpose(2, 1, 0, 3).reshape(16, 128, NFB * 128))

    in_maps = []
    for i in range(N_CORES):
        rows = slice(i * R, (i + 1) * R)
        xrT = xT[:, rows]                           # (C, R)
        xrows = np.ascontiguousarray(
            xrT.reshape(16, 128, R).transpose(1, 0, 2).reshape(128, 16 * R))
        in_maps.append({
            "xP": xP, "cosT": cosT, "sinT": sinT,
            "wqkv": wqkv_cores[i], "wproj": wproj,
            "w12": w12, "w3": w3, "xrows": xrows,
        })
    return in_maps


def _run(inputs, trace=False):
    from concourse import bass_utils
    if "nc" not in _CACHE:
        _CACHE["nc"] = _build()
    nc = _CACHE["nc"]
    in_maps = _prep_inputs(inputs)
    res = bass_utils.run_bass_kernel_spmd(
        nc, in_maps, core_ids=list(range(N_CORES)), trace=trace)
    outs = []
    for i in range(N_CORES):
        outs.append(res.results[i]["outT"].T)       # (R, C)
    full = np.concatenate(outs, axis=0)[None]       # (1, T, C)
    return np.ascontiguousarray(full.astype(np.float32)), res


def kernel(**inputs):
    out, _ = _run(inputs, trace=False)
    return out


# revision 20
# speedup vs baseline: 1.0206x; 1.0206x over previous
"""Trainium2 Bass kernel for a dense GQA transformer block (B=1, T=2048, C=2048,
16 q heads / 8 kv heads, hs=128, SwiGLU FFN=5632), SPMD across 8 NeuronCores.

Sharding: tensor-parallel attention (2 q heads + 1 kv head per core, full T),
one AllToAll to re-shard from head-parallel to row-parallel, then the attn
projection, residual, norm2 and the whole MLP run row-parallel (256 rows/core,
full weights streamed from HBM as bf16). Only collectives: two 512KB AllToAlls.

All activations are kept feature-major ("c-major" [C, T] / [HS, T]) so every
matmul maps directly onto the PE array without transposes; cross-feature
reductions (rms-norm sums, softmax sums) use ones-vector matmuls, and
partition-broadcasts use K=1 ones matmuls (reciprocals run post-broadcast on
[128, N] tiles — 1-partition DVE ops are ~8x slower).

Compute dtype: bf16 inputs to the PE with fp32 PSUM accumulation.
"""

import numpy as np
import ml_dtypes

N_CORES = 8
T = 2048
C = 2048
NH = 16
NKV = 8
HS = 128
FFN = 5632
EPS = 1e-5
R = T // N_CORES          # 256 rows (tokens) per core after the A2A
NCB = C // 128            # 16 feature blocks
NFB = FFN // 128          # 44 FFN blocks
NTCH = T // 512           # 4 T-chunks of 512
SM_SCALE = 1.0 / np.sqrt(np.float32(HS))
SM_BIAS = -10.0           # softmax exp bias; max |score| measured ~7, f32 exp safe
BF16 = ml_dtypes.bfloat16

_CACHE = {}
LDW_OPT = False


def _patch_ldw_opt():
    # bass_utils hardcodes --enable-ldw-opt=false; LDWEIGHTS at ~120ns paces
    # our N=256 matmuls. Rewrite the flag on the walrus command line.
    if not LDW_OPT or _CACHE.get("ldw_patched"):
        return
    from concourse import bass_utils
    orig = bass_utils.run_command

    def run_command(argv, **kwargs):
        argv = [a.replace("--enable-ldw-opt=false", "--enable-ldw-opt=true")
                if isinstance(a, str) else a for a in argv]
        return orig(argv, **kwargs)

    bass_utils.run_command = run_command
    _CACHE["ldw_patched"] = True


def _build():
    _patch_ldw_opt()
    import concourse.mybir as mybir
    import concourse.tile as tile
    from concourse import bacc

    f32 = mybir.dt.float32
    bf16 = mybir.dt.bfloat16
    Exp = mybir.ActivationFunctionType.Exp
    Silu = mybir.ActivationFunctionType.Silu
    Sqrt = mybir.ActivationFunctionType.Sqrt

    nc = bacc.Bacc(trn_type="TRN2", num_devices=N_CORES)

    # ---- kernel I/O (all host-pre-arranged to partition-major layouts) ----
    xP = nc.dram_tensor("xP", [128, NCB * T], bf16, kind="ExternalInput")
    cosT = nc.dram_tensor("cosT", [128, T], bf16, kind="ExternalInput")
    sinT = nc.dram_tensor("sinT", [128, T], bf16, kind="ExternalInput")
    # qkv weight tiles: [p, (db*16+cb)*128+f], db: 0=q0 1=q1 2=k 3=v
    wqkv = nc.dram_tensor("wqkv", [128, 64 * 128], bf16, kind="ExternalInput")
    # attn proj tiles per cout block: [co][p, yb*128+f]
    wproj = nc.dram_tensor("wproj", [16, 128, 16 * 128], bf16, kind="ExternalInput")
    # fc1|fc2 tiles per FFN block: [fb][p, (s*16+cb)*128+f]
    w12 = nc.dram_tensor("w12", [NFB, 128, 2 * 16 * 128], bf16, kind="ExternalInput")
    # mlp proj tiles per cout block: [co][p, fb*128+f]
    w3 = nc.dram_tensor("w3", [16, 128, NFB * 128], bf16, kind="ExternalInput")
    # residual x rows (this core's R tokens), c-major: [p, co*R+t]
    xrows = nc.dram_tensor("xrows", [128, 16 * R], f32, kind="ExternalInput")
    outT = nc.dram_tensor("outT", [C, R], f32, kind="ExternalOutput")

    with tile.TileContext(nc) as tc:
        with (
            tc.tile_pool(name="const", bufs=1) as constp,
            tc.tile_pool(name="dram", bufs=1, space="DRAM") as dramp,
            tc.tile_pool(name="w12s", bufs=7) as w12p,
            tc.tile_pool(name="qkv_acts", bufs=1) as qvp,
        ):
            # ---------------- constants ----------------
            ones_col = constp.tile([128, 1], bf16)
            nc.vector.memset(ones_col, 1.0)
            ones_row = constp.tile([1, 128], f32)
            nc.vector.memset(ones_row, 1.0)
            eps_t = constp.tile([128, 1], f32)
            nc.vector.memset(eps_t, EPS)
            smbias_t = constp.tile([128, 1], f32)
            nc.vector.memset(smbias_t, SM_BIAS)
            masks = constp.tile([128, 4 * 512], bf16)
            nc.vector.memset(masks, 1.0)
            for j in range(4):
                # keep 1 where tq >= tk + 128*j  (iota = -x + y - 128j >= 0)
                nc.gpsimd.affine_select(
                    out=masks[:, j * 512:(j + 1) * 512],
                    in_=masks[:, j * 512:(j + 1) * 512],
                    compare_op=mybir.AluOpType.is_ge,
                    fill=0.0,
                    base=-128 * j,
                    pattern=[[1, 512]],
                    channel_multiplier=-1,
                )

            # a2a buffers (one collective per local head, fired as each
            # head's attention completes -> hides trigger latency + core skew)
            a2a_in0 = dramp.tile([8 * 128, R], bf16)
            a2a_out0 = dramp.tile([8 * 128, R], bf16)
            a2a_in1 = dramp.tile([8 * 128, R], bf16)
            a2a_out1 = dramp.tile([8 * 128, R], bf16)

            qk_sb = qvp.tile([128, 3 * T], bf16)     # roped q0|q1|k, d-major
            v_sb = qvp.tile([128, NCB * 128], bf16)  # v token-major tiles

            with (
                tc.tile_pool(name="cs", bufs=1) as csp,
                tc.tile_pool(name="wqp", bufs=1) as wqpool,
                tc.tile_pool(name="xbfp", bufs=1) as xbfp,
            ):
                cs_sb = csp.tile([128, 2 * T], bf16)
                cosl = cs_sb[0:64, 0:T]
                cosh = cs_sb[64:128, 0:T]
                sinl = cs_sb[0:64, T:2 * T]
                sinh = cs_sb[64:128, T:2 * T]
                wq_sb = wqpool.tile([128, 64 * 128], bf16)
                xn = xbfp.tile([128, NCB * T], bf16)

                # ================= phase 1: rms-norm 1 =================
                with (
                    nc.named_scope("norm1"),
                    tc.tile_pool(name="xsq", bufs=3) as sqp,
                    tc.tile_pool(name="n1small", bufs=1) as n1s,
                    tc.tile_pool(name="ps_ss", bufs=1, space="PSUM") as pss,
                    tc.tile_pool(name="ps_bc", bufs=2, space="PSUM") as psb,
                ):
                    ss_ps = [pss.tile([1, 512], f32, name=f"ss{t4}", tag=f"ss{t4}")
                             for t4 in range(NTCH)]
                    for cb in range(NCB):
                        xsl = xn[:, cb * T:(cb + 1) * T]
                        nc.sync.dma_start(xsl, xP[:, cb * T:(cb + 1) * T])
                        xsq = sqp.tile([128, T], bf16, tag="xsq")
                        nc.vector.tensor_mul(xsq[:], xsl, xsl)
                        for t4 in range(NTCH):
                            nc.tensor.matmul(
                                ss_ps[t4][:], ones_col[:],
                                xsq[:, t4 * 512:(t4 + 1) * 512],
                                start=(cb == 0), stop=(cb == NCB - 1))
                    sg = n1s.tile([1, T], f32)
                    rbc = n1s.tile([128, T], bf16)
                    for t4 in range(NTCH):
                        ch = slice(t4 * 512, (t4 + 1) * 512)
                        nc.scalar.activation(sg[:, ch], ss_ps[t4][:],
                                             Sqrt, bias=eps_t[0:1, :], scale=1.0 / C)
                        bc = psb.tile([128, 512], f32, tag="bc")
                        nc.tensor.matmul(bc[:], ones_row[:], sg[:, ch],
                                         start=True, stop=True)
                        rtmp = n1s.tile([128, 512], f32, tag="rtmp", bufs=2)
                        nc.vector.reciprocal_approx_fast(out=rtmp[:], in_=bc[:])
                        nc.vector.tensor_copy(rbc[:, ch], rtmp[:])
                    for cb in range(NCB):
                        nc.vector.tensor_mul(xn[:, cb * T:(cb + 1) * T],
                                             xn[:, cb * T:(cb + 1) * T], rbc[:])

                # qkv weights + rope tables load behind the x stream
                for db in range(4):
                    nc.sync.dma_start(wq_sb[:, db * 2048:(db + 1) * 2048],
                                      wqkv[:, db * 2048:(db + 1) * 2048])
                nc.sync.dma_start(cs_sb[:, 0:T], cosT[:])
                nc.sync.dma_start(cs_sb[:, T:2 * T], sinT[:])

                # ================= phase 2: qkv + rope =================
                with (
                    nc.named_scope("qkv"),
                    tc.tile_pool(name="ropetmp", bufs=6) as rtp,
                    tc.tile_pool(name="ps_qk", bufs=3, space="PSUM") as psqk,
                    tc.tile_pool(name="ps_v", bufs=2, space="PSUM") as psv,
                ):
                    for db in range(3):  # q0, q1, k -> d-major
                        for t4 in range(NTCH):
                            qp = psqk.tile([128, 512], f32, tag="qk")
                            for cb in range(NCB):
                                nc.tensor.matmul(
                                    qp[:],
                                    wq_sb[:, (db * 16 + cb) * 128:
                                          (db * 16 + cb + 1) * 128],
                                    xn[:, cb * T + t4 * 512: cb * T + (t4 + 1) * 512],
                                    start=(cb == 0), stop=(cb == NCB - 1))
                            # rope into qk_sb[:, db*T + chunk]. SBUF-SBUF DVE ops
                            # need equal base partitions; PSUM inputs are exempt,
                            # so crossed-half terms read q straight from PSUM.
                            ch = slice(t4 * 512, (t4 + 1) * 512)
                            dst = qk_sb[:, db * T + t4 * 512: db * T + (t4 + 1) * 512]
                            rc = rtp.tile([128, 512], bf16, tag="rc")
                            nc.vector.tensor_mul(rc[:], qp[:], cs_sb[:, 0:T][:, ch])
                            cross = rtp.tile([128, 512], bf16, tag="cross")
                            nc.vector.tensor_mul(cross[0:64, :], qp[64:128, :],
                                                 sinl[:, ch])
                            nc.vector.tensor_mul(cross[64:128, :], qp[0:64, :],
                                                 sinh[:, ch])
                            nc.vector.tensor_sub(dst[0:64, :], rc[0:64, :],
                                                 cross[0:64, :])
                            nc.vector.tensor_add(dst[64:128, :], rc[64:128, :],
                                                 cross[64:128, :])
                    for tb_ in range(NCB):  # v token-major
                        vp = psv.tile([128, 128], f32, tag="v")
                        for cb in range(NCB):
                            nc.tensor.matmul(
                                vp[:],
                                xn[:, cb * T + tb_ * 128: cb * T + (tb_ + 1) * 128],
                                wq_sb[:, (48 + cb) * 128:(48 + cb + 1) * 128],
                                start=(cb == 0), stop=(cb == NCB - 1))
                        nc.vector.tensor_copy(
                            v_sb[:, tb_ * 128:(tb_ + 1) * 128], vp[:])

            # x / cos / qkv-weight buffers freed here; MLP+proj weights stream in.
            # allocate all MLP fc weight tiles now so their DMAs start during attn
            w12_tiles = {}
            for fb in range(NFB):
                w12_tiles[fb] = w12p.tile(
                    [128, 2 * 16 * 128], bf16, name=f"w12t{fb}", tag="w12t")
                nc.sync.dma_start(w12_tiles[fb][:], w12[fb])

            with (
                tc.tile_pool(name="wprojs", bufs=6) as projp,
                tc.tile_pool(name="late", bufs=1) as latep,
            ):
                proj_tiles = []
                for co in range(16):
                    wt = projp.tile([128, 16 * 128], bf16, name=f"projw{co}",
                                    tag="projw")
                    nc.sync.dma_start(wt[:], wproj[co])
                    proj_tiles.append(wt)

                x2_sb = latep.tile([128, 16 * R], f32)
                xn2_sb = latep.tile([128, 16 * R], bf16)
                h_sb = latep.tile([128, NFB * R], bf16)
                y_all = latep.tile([128, 16 * R], bf16)

                # ============ phase 3: attention (2 heads per core) ============
                with (
                    nc.named_scope("attn"),
                    tc.tile_pool(name="pp_p", bufs=10) as ppool,
                    tc.tile_pool(name="pp_y", bufs=4) as ypool,
                    tc.tile_pool(name="attn_small", bufs=4) as asml,
                    tc.tile_pool(name="ps_s", bufs=3, space="PSUM") as ps_s,
                    tc.tile_pool(name="ps_y", bufs=2, space="PSUM") as ps_y,
                    tc.tile_pool(name="ps_sum", bufs=2, space="PSUM") as ps_sum,
                    tc.tile_pool(name="ps_abc", bufs=1, space="PSUM") as ps_abc,
                ):
                    for h in range(2):
                        a2a_in_h = a2a_in0 if h == 0 else a2a_in1
                        q_ap = qk_sb[:, h * T:(h + 1) * T]
                        k_ap = qk_sb[:, 2 * T:3 * T]
                        for qi in range(NTCH):
                            nkb = 4 * qi + 4
                            yp = ps_y.tile([128, 512], f32, tag="y")
                            sacc = asml.tile([128, 512], f32, tag="sacc")
                            pend = []  # 2-deep SW pipeline: AV trails scores
                            ptiles = {}

                            def flush(kb):
                                ppt = ptiles[kb]
                                nc.tensor.matmul(
                                    yp[:], v_sb[:, kb * 128:(kb + 1) * 128],
                                    ppt[:], start=(kb == 0), stop=(kb == nkb - 1))
                                if kb == 0:
                                    nc.vector.tensor_copy(sacc[:], ppt[:])
                                else:
                                    nc.vector.tensor_add(sacc[:], sacc[:], ppt[:])

                            for kb in range(nkb):
                                sp = ps_s.tile([128, 512], f32, tag="s")
                                nc.tensor.matmul(
                                    sp[:], k_ap[:, kb * 128:(kb + 1) * 128],
                                    q_ap[:, qi * 512:(qi + 1) * 512],
                                    start=True, stop=True)
                                pt = ppool.tile([128, 512], bf16, tag="p")
                                nc.scalar.activation(pt[:], sp[:], Exp,
                                                     bias=smbias_t[:],
                                                     scale=float(SM_SCALE))
                                if kb >= 4 * qi:
                                    moff = kb - 4 * qi
                                    nc.vector.tensor_mul(
                                        pt[:], pt[:],
                                        masks[:, moff * 512:(moff + 1) * 512])
                                ptiles[kb] = pt
                                pend.append(kb)
                                if len(pend) > 3:
                                    flush(pend.pop(0))
                            while pend:
                                flush(pend.pop(0))
                            # S = column-sums of accumulated p, then 1/S bcast
                            sump = ps_sum.tile([1, 512], f32, tag="sum")
                            sacc_bf = asml.tile([128, 512], bf16, tag="saccb")
                            nc.vector.tensor_copy(sacc_bf[:], sacc[:])
                            nc.tensor.matmul(sump[:], ones_col[:], sacc_bf[:],
                                             start=True, stop=True)
                            ssb = asml.tile([1, 512], f32, tag="ssb")
                            nc.vector.tensor_copy(ssb[:], sump[:])
                            bcp = ps_abc.tile([128, 512], f32, tag="abc")
                            nc.tensor.matmul(bcp[:], ones_row[:], ssb[:],
                                             start=True, stop=True)
                            bsb = asml.tile([128, 512], f32, tag="bsb")
                            nc.vector.reciprocal_approx_fast(out=bsb[:], in_=bcp[:])
                            ysb = ypool.tile([128, 512], bf16, tag="ysb")
                            nc.vector.tensor_mul(ysb[:], yp[:], bsb[:])
                            # scatter two 256-token halves to this head's A2A buf
                            for half in range(2):
                                g = 2 * qi + half
                                nc.sync.dma_start(
                                    a2a_in_h[128 * g: 128 * (g + 1), :],
                                    ysb[:, half * 256:(half + 1) * 256])
                        # fire this head's A2A as soon as its outputs are staged,
                        # and pull the result into SBUF immediately
                        a2a_out_h = a2a_out0 if h == 0 else a2a_out1
                        nc.gpsimd.collective_compute(
                            "AllToAll", mybir.AluOpType.bypass,
                            replica_groups=[list(range(N_CORES))],
                            ins=[a2a_in_h.opt()], outs=[a2a_out_h.opt()])
                        for g in range(8):
                            nc.sync.dma_start(
                                y_all[:, (h * 8 + g) * R:(h * 8 + g + 1) * R],
                                a2a_out_h[g * 128:(g + 1) * 128, :])

                # ======== phase 5: proj + residual + norm2 (row-local) ========
                with (
                    nc.named_scope("proj"),
                    tc.tile_pool(name="sq2", bufs=2) as sq2p,
                    tc.tile_pool(name="xrow", bufs=1) as xrp,
                    tc.tile_pool(name="n2small", bufs=1) as n2s,
                    tc.tile_pool(name="ps_acc", bufs=2, space="PSUM") as psa,
                    tc.tile_pool(name="ps_ss2", bufs=1, space="PSUM") as pss2,
                    tc.tile_pool(name="ps_bc2", bufs=1, space="PSUM") as psb2,
                ):
                    xr_sb = xrp.tile([128, 16 * R], f32)
                    nc.scalar.dma_start(xr_sb[:], xrows[:])
                    ss2 = pss2.tile([1, R], f32, tag="ss2")
                    for co in range(16):
                        wt = proj_tiles[co]
                        ap = psa.tile([128, R], f32, tag="acc")
                        for yb in range(16):
                            nc.tensor.matmul(ap[:], wt[:, yb * 128:(yb + 1) * 128],
                                             y_all[:, yb * R:(yb + 1) * R],
                                             start=(yb == 0), stop=(yb == 15))
                        cs_ = slice(co * R, (co + 1) * R)
                        nc.vector.tensor_add(x2_sb[:, cs_], ap[:], xr_sb[:, cs_])
                        xq2 = sq2p.tile([128, R], bf16, tag="xq2")
                        nc.vector.tensor_mul(xq2[:], x2_sb[:, cs_], x2_sb[:, cs_])
                        nc.tensor.matmul(ss2[:], ones_col[:], xq2[:],
                                         start=(co == 0), stop=(co == 15))
                    sg2 = n2s.tile([1, R], f32)
                    nc.scalar.activation(sg2[:], ss2[:], Sqrt,
                                         bias=eps_t[0:1, :], scale=1.0 / C)
                    bc2 = psb2.tile([128, R], f32, tag="bc2")
                    nc.tensor.matmul(bc2[:], ones_row[:], sg2[:],
                                     start=True, stop=True)
                    b2sb = n2s.tile([128, R], f32)
                    nc.vector.reciprocal_approx_fast(out=b2sb[:], in_=bc2[:])
                    for co in range(16):
                        cs_ = slice(co * R, (co + 1) * R)
                        nc.vector.tensor_mul(xn2_sb[:, cs_], x2_sb[:, cs_], b2sb[:])

                # ================= phase 6a: MLP fc1/fc2 + swiglu =============
                with (
                    tc.tile_pool(name="w3s", bufs=3) as w3p,
                ):
                    w3_tiles = []
                    for co in range(16):
                        w3t = w3p.tile([128, NFB * 128], bf16, name=f"w3t{co}",
                                       tag="w3w")
                        nc.sync.dma_start(w3t[:], w3[co])
                        w3_tiles.append(w3t)
                    with (
                        nc.named_scope("mlp_fc"),
                        tc.tile_pool(name="hsil", bufs=2) as hsp,
                        tc.tile_pool(name="ps_h1", bufs=2, space="PSUM") as psh1,
                        tc.tile_pool(name="ps_h2", bufs=2, space="PSUM") as psh2,
                    ):
                        for fb in range(NFB):
                            wt = w12_tiles[fb]
                            h1 = psh1.tile([128, R], f32, tag="h1")
                            h2 = psh2.tile([128, R], f32, tag="h2")
                            for cb in range(16):
                                nc.tensor.matmul(
                                    h1[:], wt[:, cb * 128:(cb + 1) * 128],
                                    xn2_sb[:, cb * R:(cb + 1) * R],
                                    start=(cb == 0), stop=(cb == 15))
                            for cb in range(16):
                                nc.tensor.matmul(
                                    h2[:], wt[:, (16 + cb) * 128:(17 + cb) * 128],
                                    xn2_sb[:, cb * R:(cb + 1) * R],
                                    start=(cb == 0), stop=(cb == 15))
                            hs = hsp.tile([128, R], f32, tag="hs")
                            nc.scalar.activation(hs[:], h1[:], Silu)
                            nc.vector.tensor_mul(h_sb[:, fb * R:(fb + 1) * R],
                                                 hs[:], h2[:])

                    # ============== phase 6b: MLP proj + final residual =======
                    with (
                        nc.named_scope("mlp_proj"),
                        tc.tile_pool(name="outp", bufs=3) as outp,
                        tc.tile_pool(name="ps_o", bufs=2, space="PSUM") as pso,
                    ):
                        for co in range(16):
                            w3t = w3_tiles[co]
                            op = pso.tile([128, R], f32, tag="o")
                            for fb in range(NFB):
                                nc.tensor.matmul(
                                    op[:], w3t[:, fb * 128:(fb + 1) * 128],
                                    h_sb[:, fb * R:(fb + 1) * R],
                                    start=(fb == 0), stop=(fb == NFB - 1))
                            osb = outp.tile([128, R], f32, tag="osb")
                            nc.vector.tensor_add(osb[:], op[:],
                                                 x2_sb[:, co * R:(co + 1) * R])
                            nc.scalar.dma_start(outT[co * 128:(co + 1) * 128, :], osb[:])

    nc.compile()
    return nc


def _prep_inputs(inputs):
    """Host-side sharding / layout / dtype prep. Returns per-core in_maps."""
    x = np.asarray(inputs["x"], np.float32)[0]        # (T, C)
    cos = np.asarray(inputs["cos"], np.float32)[0]    # (T, HS)
    sin = np.asarray(inputs["sin"], np.float32)[0]
    qkv_w = np.asarray(inputs["qkv_w"], np.float32)   # (4096, C)
    proj_w = np.asarray(inputs["proj_w"], np.float32)  # (C, 2048)
    fc1_w = np.asarray(inputs["fc1_w"], np.float32)   # (FFN, C)
    fc2_w = np.asarray(inputs["fc2_w"], np.float32)
    mlp_proj_w = np.asarray(inputs["mlp_proj_w"], np.float32)  # (C, FFN)
    n1 = np.asarray(inputs["norm1_w"], np.float32)
    n2 = np.asarray(inputs["norm2_w"], np.float32)

    xT = np.ascontiguousarray(x.T)                    # (C, T)
    # xP[p, cb*T + t] = xT[cb*128+p, t]
    xP = np.ascontiguousarray(
        xT.reshape(NCB, 128, T).transpose(1, 0, 2).reshape(128, NCB * T)).astype(BF16)
    cosT = np.ascontiguousarray(cos.T).astype(BF16)
    sinT = np.ascontiguousarray(sin.T).astype(BF16)

    qkv_eff = (qkv_w * n1[None, :]).astype(BF16)      # fold norm1 weight
    # per-core d-major weight tiles
    wqkv_cores = []
    for i in range(N_CORES):
        dblocks = [
            qkv_eff[(2 * i) * HS:(2 * i + 1) * HS],       # q0
            qkv_eff[(2 * i + 1) * HS:(2 * i + 2) * HS],   # q1
            qkv_eff[NH * HS + i * HS: NH * HS + (i + 1) * HS],            # k
            qkv_eff[(NH + NKV) * HS + i * HS: (NH + NKV) * HS + (i + 1) * HS],  # v
        ]
        # tile (db, cb): lhsT[p, f] = W^T[cb*128+p, db*128+f] = W[db*128+f, cb*128+p]
        blocks = [dblocks[db].T.reshape(NCB, 128, 128) for db in range(4)]
        arr = np.stack(blocks, axis=0)              # (db, cb, p, f)
        wqkv_cores.append(np.ascontiguousarray(
            arr.transpose(2, 0, 1, 3).reshape(128, 64 * 128)))

    projT = proj_w.T.astype(BF16)                   # (ych, cout)
    # y_all channel-block order after the two per-head A2As: blocks 0-7 are the
    # even global heads (local head 0 of cores 0-7), 8-15 the odd ones.
    perm = [2 * g for g in range(8)] + [2 * g + 1 for g in range(8)]
    wproj = np.ascontiguousarray(
        projT.reshape(16, 128, 16, 128)[perm].transpose(2, 1, 0, 3)
        .reshape(16, 128, 16 * 128))

    w1T = (fc1_w * n2[None, :]).T.astype(BF16)      # (C, FFN)
    w2T = (fc2_w * n2[None, :]).T.astype(BF16)
    # w12[fb][p, (s*16+cb)*128+f] = wsT[cb*128+p, fb*128+f]
    a1 = w1T.reshape(NCB, 128, NFB, 128)            # (cb, p, fb, f)
    a2 = w2T.reshape(NCB, 128, NFB, 128)
    w12 = np.ascontiguousarray(
        np.stack([a1, a2], axis=0)                  # (s, cb, p, fb, f)
        .transpose(3, 2, 0, 1, 4)                   # (fb, p, s, cb, f)
        .reshape(NFB, 128, 2 * 16 * 128))
    mlpT = mlp_proj_w.T.astype(BF16)                # (FFN, C)
    w3 = np.ascontiguousarray(
        mlpT.reshape(NFB, 128, 16, 128).transpose(2, 1, 0, 3).reshape(16, 128, NFB * 128))

    in_maps = []
    for i in range(N_CORES):
        rows = slice(i * R, (i + 1) * R)
        xrT = xT[:, rows]                           # (C, R)
        xrows = np.ascontiguousarray(
            xrT.reshape(16, 128, R).transpose(1, 0, 2).reshape(128, 16 * R))
        in_maps.append({
            "xP": xP, "cosT": cosT, "sinT": sinT,
            "wqkv": wqkv_cores[i], "wproj": wproj,
            "w12": w12, "w3": w3, "xrows": xrows,
        })
    return in_maps


def _run(inputs, trace=False):
    from concourse import bass_utils
    if "nc" not in _CACHE:
        _CACHE["nc"] = _build()
    nc = _CACHE["nc"]
    in_maps = _prep_inputs(inputs)
    res = bass_utils.run_bass_kernel_spmd(
        nc, in_maps, core_ids=list(range(N_CORES)), trace=trace)
    outs = []
    for i in range(N_CORES):
        outs.append(res.results[i]["outT"].T)       # (R, C)
    full = np.concatenate(outs, axis=0)[None]       # (1, T, C)
    return np.ascontiguousarray(full.astype(np.float32)), res


def kernel(**inputs):
    out, _ = _run(inputs, trace=False)
    return out
